# revision 10
# baseline (speedup 1.0000x reference)
"""Trainium2 Bass kernel: 3x3 same-padding Conv2D, NCHW.

Input  (16, 64, 128, 128) f32, weights (128, 64, 3, 3) OIHW, bias (128,).
Output (16, 128, 128, 128) f32.  8 NeuronCores, 2 images per core.

Strategy (v2, image-pair packing):
  - The two images of a core share the 128 SBUF partitions: partitions
    0-63 hold img0's 64 input channels (zero-padded to 130x130),
    partitions 64-127 hold img1's.  No data duplication: input DMA is
    4.3 MB/core fp16 (the v1 dual layout moved 8.7 MB).
  - Every conv tap (kh, kw) is a K=64 matmul; the img0 tap (partitions
    0-63, PSUM bank A) and img1 tap (partitions 64-127, bank B) are
    issued adjacently so the PE runs them concurrently on disjoint
    row-group halves -> 1 effective slot per tap, the K=128 ideal.
  - Slab = 8 output rows of both images = 4 PSUM banks; per slab the 9
    taps are 9x4 matmuls = 18 pair-slots (9 per 8 rows, ideal).
  - Epilogue: ScalarE and VectorE each bias-add two banks into an fp16
    [128, 2048] tile laid out [r, img, w]; ONE contiguous 512 KB store
    per slab (scalar HWDGE ring).  Output DRAM layout is [cout, h, img,
    w]; the host transposes to [img, cout, h, w] and upcasts to f32
    (tolerance is 2e-2; fp16 output rounding is ~5e-4).
  - Input is DMA'd in 9 row-chunks on the sync ring so compute starts
    after the first 10 rows land.

Every instruction may carry at most ONE semaphore wait on this
toolchain -- bacc.Bacc's compile() pipeline enforces that, which is why
this builds a Bacc, not a raw bass.Bass.
"""

import sys

if "/opt/trn_rl_repo" not in sys.path:
    sys.path.insert(0, "/opt/trn_rl_repo")

import numpy as np

N_CORES = 8
IMGS_PER_CORE = 2
H = 128
W = 128
CIN = 64
COUT = 128
WPAD = W + 2  # 130: one zero column each side
HPAD = H + 2  # 130 rows (pad row above and below)
ROWS_PER_BANK = 4   # 4*128 = 512 f32 = one PSUM bank
ROWS_PER_SLAB = 8   # 2 banks per image, 4 banks per slab
N_TAPS = 9

_cache = {}


def _build_nc():
    import concourse.mybir as mybir
    from concourse import bacc
    from concourse.tile import TileContext

    f32 = mybir.dt.float32
    f16 = mybir.dt.float16

    nc = bacc.Bacc(target_bir_lowering=False)
    # partitions 0-63: img0 padded channels; 64-127: img1
    x_d = nc.dram_tensor("x", [128, HPAD * WPAD], f16, kind="ExternalInput")
    # w[tap] duplicated on both partition halves: wb[p, t*128+co]
    wb_d = nc.dram_tensor("wb", [128, N_TAPS * COUT], f16, kind="ExternalInput")
    b_d = nc.dram_tensor("b", [COUT, 1], f32, kind="ExternalInput")
    # [cout, h, img, w] fp16; host transposes to [img, cout, h, w] + f32
    out_d = nc.dram_tensor(
        "out", [COUT, H * IMGS_PER_CORE * W], f16, kind="ExternalOutput"
    )

    with TileContext(nc) as tc:
        with (
            tc.tile_pool(name="wpool", bufs=1) as wpool,
            tc.tile_pool(name="xpool", bufs=1) as xpool,
            tc.tile_pool(name="opool", bufs=3) as opool,
            tc.tile_pool(name="pspool", bufs=2, space="PSUM") as pspool,
        ):
            wb_sb = wpool.tile([128, N_TAPS * COUT], f16)
            nc.sync.dma_start(out=wb_sb[:], in_=wb_d[:])
            b_f32 = wpool.tile([COUT, 1], f32)
            b_sb = b_f32[:]

            X = xpool.tile([128, HPAD * WPAD], f16)
            # first 6 rows ride the scalar ring, in parallel with the
            # weight DMA on sync, so the first matmul can start ASAP;
            # the rest stream on sync.  bias follows on scalar (it is
            # not needed until the first evacuation).
            edges = [0, 6, 22, 38, 54, 70, 86, 102, 118, HPAD]
            for i, (r0, r1) in enumerate(zip(edges[:-1], edges[1:])):
                eng = nc.scalar if i == 0 else nc.sync
                eng.dma_start(
                    out=X[:, r0 * WPAD : r1 * WPAD],
                    in_=x_d[:, r0 * WPAD : r1 * WPAD],
                )
                if i == 0:
                    nc.scalar.dma_start(out=b_f32[:], in_=b_d[:])
            X3 = X.rearrange("p (r c) -> p r c", c=WPAD)

            # HAM warm-up: junk matmuls on an uninitialized scratch tile
            # (no input dependencies, so they issue right after the
            # engine-sync preamble, while the first DMAs are still in
            # flight).  PE activity starts ~2 us earlier, so the
            # activity monitor un-throttles the PE clock (1.2 ->
            # 2.4 GHz) before the real work arrives.  Results land in a
            # PSUM bank that slab 1 later overwrites with start=True.
            junk_src = wpool.tile([128, ROWS_PER_BANK * W], f16)
            nc.vector.memset(junk_src[:], 0)
            warm = pspool.tile([COUT, ROWS_PER_BANK * W], f32, tag="psA0")
            for _ in range(3):
                nc.tensor.matmul(
                    warm[:],
                    junk_src[:, 0:COUT],
                    junk_src[:],
                    start=True,
                    stop=True,
                )

            for s in range(H // ROWS_PER_SLAB):
                h0 = s * ROWS_PER_SLAB
                h1 = h0 + ROWS_PER_BANK
                psA0 = pspool.tile([COUT, ROWS_PER_BANK * W], f32, tag="psA0")
                psB0 = pspool.tile([COUT, ROWS_PER_BANK * W], f32, tag="psB0")
                psA1 = pspool.tile([COUT, ROWS_PER_BANK * W], f32, tag="psA1")
                psB1 = pspool.tile([COUT, ROWS_PER_BANK * W], f32, tag="psB1")
                last = s == H // ROWS_PER_SLAB - 1
                for t in range(N_TAPS):
                    kh, kw = divmod(t, 3)
                    lo = wb_sb[0:CIN, t * COUT : (t + 1) * COUT]
                    hi = wb_sb[CIN:128, t * COUT : (t + 1) * COUT]
                    st = t == 0
                    sp = t == N_TAPS - 1
                    # adjacent lo/hi matmuls run concurrently on disjoint
                    # PE row-group halves (different PSUM banks).  The
                    # last slab runs group 1 first so its banks finish
                    # early and the epilogue/store pipeline drains sooner.
                    pairs = [
                        (psA0, psB0, h0),
                        (psA1, psB1, h1),
                    ]
                    if last:
                        pairs.reverse()
                    for psA, psB, h in pairs:
                        nc.tensor.matmul(
                            psA[:],
                            lo,
                            X3[0:CIN, h + kh : h + kh + ROWS_PER_BANK, kw : kw + W],
                            start=st,
                            stop=sp,
                        )
                        nc.tensor.matmul(
                            psB[:],
                            hi,
                            X3[CIN:128, h + kh : h + kh + ROWS_PER_BANK, kw : kw + W],
                            start=st,
                            stop=sp,
                        )
                # bias-add into fp16 tile, layout [r(8), img(2), w(128)];
                # ScalarE takes the first 4 rows, VectorE the last 4
                ob = opool.tile([COUT, ROWS_PER_SLAB * IMGS_PER_CORE * W], f16)
                obv = ob.rearrange("p (r i c) -> p r i c", i=IMGS_PER_CORE, c=W)
                psA0v = psA0.rearrange("p (r c) -> p r c", c=W)
                psB0v = psB0.rearrange("p (r c) -> p r c", c=W)
                psA1v = psA1.rearrange("p (r c) -> p r c", c=W)
                psB1v = psB1.rearrange("p (r c) -> p r c", c=W)
                if s < H // ROWS_PER_SLAB - 1:
                    nc.scalar.add(obv[:, 0:4, 0, :], psA0v[:], b_sb)
                    nc.scalar.add(obv[:, 0:4, 1, :], psB0v[:], b_sb)
                    nc.vector.tensor_scalar_add(obv[:, 4:8, 0, :], psA1v[:], b_sb)
                    nc.vector.tensor_scalar_add(obv[:, 4:8, 1, :], psB1v[:], b_sb)
                    # one contiguous 512 KB store per slab
                    nc.scalar.dma_start(
                        out=out_d[
                            :,
                            h0 * IMGS_PER_CORE * W : (h0 + ROWS_PER_SLAB)
                            * IMGS_PER_CORE
                            * W,
                        ],
                        in_=ob[:],
                    )
                else:
                    # last slab: group 1 ran first (pairs reversed), so
                    # its banks are done ~0.4us before group 0's.  Both
                    # engines evacuate group 1 in parallel, its 256 KB
                    # store goes out on the idle sync ring, then group 0
                    # follows -- the final receipt round-trip overlaps
                    # the previous store's transfer.
                    half = ROWS_PER_BANK * IMGS_PER_CORE * W
                    nc.scalar.add(obv[:, 4:8, 0, :], psA1v[:], b_sb)
                    nc.vector.tensor_scalar_add(obv[:, 4:8, 1, :], psB1v[:], b_sb)
                    nc.sync.dma_start(
                        out=out_d[:, h0 * IMGS_PER_CORE * W + half :][:, 0:half],
                        in_=ob[:, half : 2 * half],
                    )
                    nc.scalar.add(obv[:, 0:4, 0, :], psA0v[:], b_sb)
                    nc.vector.tensor_scalar_add(obv[:, 0:4, 1, :], psB0v[:], b_sb)
                    nc.sync.dma_start(
                        out=out_d[:, h0 * IMGS_PER_CORE * W :][:, 0:half],
                        in_=ob[:, 0:half],
                    )
    nc.compile()
    return nc


def _get_nc():
    if "nc" not in _cache:
        _cache["nc"] = _build_nc()
    return _cache["nc"]


def _prepare_in_maps(input_tensor, weights, bias):
    input_tensor = np.asarray(input_tensor, dtype=np.float32)
    weights = np.asarray(weights, dtype=np.float32)
    bias = np.asarray(bias, dtype=np.float32)
    # wb[ci, t*128+co] = W[co, ci, kh, kw], t = kh*3+kw; both halves
    w9 = weights.transpose(1, 2, 3, 0).reshape(CIN, N_TAPS * COUT)  # ci,(kh kw co)
    wb = np.empty((128, N_TAPS * COUT), dtype=np.float16)
    wb[0:CIN] = w9
    wb[CIN:128] = w9
    wb = np.ascontiguousarray(wb)
    b = np.ascontiguousarray(bias.reshape(COUT, 1))
    in_maps = []
    for c in range(N_CORES):
        imgs = input_tensor[c * IMGS_PER_CORE : (c + 1) * IMGS_PER_CORE]
        zp = np.zeros((IMGS_PER_CORE, CIN, HPAD, WPAD), dtype=np.float16)
        zp[:, :, 1 : H + 1, 1 : W + 1] = imgs
        shard = np.ascontiguousarray(zp.reshape(128, HPAD * WPAD))
        in_maps.append({"x": shard, "wb": wb, "b": b})
    return in_maps


def _gather(results):
    outs = []
    for c in range(N_CORES):
        o = results[c]["out"].reshape(COUT, H, IMGS_PER_CORE, W)
        outs.append(np.ascontiguousarray(o.transpose(2, 0, 1, 3), dtype=np.float32))
    return np.concatenate(outs, axis=0)


def kernel(input_tensor, weights, bias):
    from concourse.bass_utils import run_bass_kernel_spmd

    nc = _get_nc()
    in_maps = _prepare_in_maps(input_tensor, weights, bias)
    res = run_bass_kernel_spmd(nc, in_maps, core_ids=list(range(N_CORES)))
    return _gather(res.results)


# revision 11
# speedup vs baseline: 1.0354x; 1.0354x over previous
"""Trainium2 Bass kernel: 3x3 same-padding Conv2D, NCHW.

Input  (16, 64, 128, 128) f32, weights (128, 64, 3, 3) OIHW, bias (128,).
Output (16, 128, 128, 128) f32.  8 NeuronCores, 2 images per core.

Strategy (v2, image-pair packing):
  - The two images of a core share the 128 SBUF partitions: partitions
    0-63 hold img0's 64 input channels (zero-padded to 130x130),
    partitions 64-127 hold img1's.  No data duplication: input DMA is
    4.3 MB/core fp16 (the v1 dual layout moved 8.7 MB).
  - Every conv tap (kh, kw) is a K=64 matmul; the img0 tap (partitions
    0-63, PSUM bank A) and img1 tap (partitions 64-127, bank B) are
    issued adjacently so the PE runs them concurrently on disjoint
    row-group halves -> 1 effective slot per tap, the K=128 ideal.
  - Slab = 8 output rows of both images = 4 PSUM banks; per slab the 9
    taps are 9x4 matmuls = 18 pair-slots (9 per 8 rows, ideal).
  - Epilogue: ScalarE and VectorE each bias-add two banks into an fp16
    [128, 2048] tile laid out [r, img, w]; ONE contiguous 512 KB store
    per slab (scalar HWDGE ring).  Output DRAM layout is [cout, h, img,
    w]; the host transposes to [img, cout, h, w] and upcasts to f32
    (tolerance is 2e-2; fp16 output rounding is ~5e-4).
  - Input is DMA'd in 9 row-chunks on the sync ring so compute starts
    after the first 10 rows land.

Every instruction may carry at most ONE semaphore wait on this
toolchain -- bacc.Bacc's compile() pipeline enforces that, which is why
this builds a Bacc, not a raw bass.Bass.
"""

import sys

if "/opt/trn_rl_repo" not in sys.path:
    sys.path.insert(0, "/opt/trn_rl_repo")

import numpy as np

N_CORES = 8
IMGS_PER_CORE = 2
H = 128
W = 128
CIN = 64
COUT = 128
WPAD = W + 2  # 130: one zero column each side
HPAD = H + 2  # 130 rows (pad row above and below)
ROWS_PER_BANK = 4   # 4*128 = 512 f32 = one PSUM bank
ROWS_PER_SLAB = 8   # 2 banks per image, 4 banks per slab
N_TAPS = 9

_cache = {}


def _build_nc():
    import concourse.mybir as mybir
    from concourse import bacc
    from concourse.tile import TileContext

    f32 = mybir.dt.float32
    f16 = mybir.dt.float16

    nc = bacc.Bacc(target_bir_lowering=False)
    # partitions 0-63: img0 padded channels; 64-127: img1
    x_d = nc.dram_tensor("x", [128, HPAD * WPAD], f16, kind="ExternalInput")
    # w[tap] duplicated on both partition halves: wb[p, t*128+co]
    wb_d = nc.dram_tensor("wb", [128, N_TAPS * COUT], f16, kind="ExternalInput")
    b_d = nc.dram_tensor("b", [COUT, 1], f32, kind="ExternalInput")
    # [cout, h, img, w] fp16; host transposes to [img, cout, h, w] + f32
    out_d = nc.dram_tensor(
        "out", [COUT, H * IMGS_PER_CORE * W], f16, kind="ExternalOutput"
    )

    with TileContext(nc) as tc:
        with (
            tc.tile_pool(name="wpool", bufs=1) as wpool,
            tc.tile_pool(name="xpool", bufs=1) as xpool,
            tc.tile_pool(name="opool", bufs=3) as opool,
            tc.tile_pool(name="pspool", bufs=2, space="PSUM") as pspool,
        ):
            wb_sb = wpool.tile([128, N_TAPS * COUT], f16)
            nc.sync.dma_start(out=wb_sb[:], in_=wb_d[:])
            b_f32 = wpool.tile([COUT, 1], f32)
            b_sb = b_f32[:]

            X = xpool.tile([128, HPAD * WPAD], f16)
            # first 6 rows ride the scalar ring, in parallel with the
            # weight DMA on sync, so the first matmul can start ASAP;
            # the rest stream on sync.  bias follows on scalar (it is
            # not needed until the first evacuation).
            edges = [0, 6, 22, 38, 54, 70, 86, 102, 118, HPAD]
            for i, (r0, r1) in enumerate(zip(edges[:-1], edges[1:])):
                eng = nc.scalar if i == 0 else nc.sync
                eng.dma_start(
                    out=X[:, r0 * WPAD : r1 * WPAD],
                    in_=x_d[:, r0 * WPAD : r1 * WPAD],
                )
                if i == 0:
                    nc.scalar.dma_start(out=b_f32[:], in_=b_d[:])
            X3 = X.rearrange("p (r c) -> p r c", c=WPAD)

            # HAM warm-up: junk matmuls on an uninitialized scratch tile
            # (no input dependencies, so they issue right after the
            # engine-sync preamble, while the first DMAs are still in
            # flight).  PE activity starts ~2 us earlier, so the
            # activity monitor un-throttles the PE clock (1.2 ->
            # 2.4 GHz) before the real work arrives.  Results land in a
            # PSUM bank that slab 1 later overwrites with start=True.
            junk_src = wpool.tile([128, ROWS_PER_BANK * W], f16)
            nc.vector.memset(junk_src[:], 0)
            warm = pspool.tile([COUT, ROWS_PER_BANK * W], f32, tag="psA0")
            for _ in range(4):
                nc.tensor.matmul(
                    warm[:],
                    junk_src[:, 0:COUT],
                    junk_src[:],
                    start=True,
                    stop=True,
                )

            for s in range(H // ROWS_PER_SLAB):
                h0 = s * ROWS_PER_SLAB
                h1 = h0 + ROWS_PER_BANK
                psA0 = pspool.tile([COUT, ROWS_PER_BANK * W], f32, tag="psA0")
                psB0 = pspool.tile([COUT, ROWS_PER_BANK * W], f32, tag="psB0")
                psA1 = pspool.tile([COUT, ROWS_PER_BANK * W], f32, tag="psA1")
                psB1 = pspool.tile([COUT, ROWS_PER_BANK * W], f32, tag="psB1")
                last = s == H // ROWS_PER_SLAB - 1
                for t in range(N_TAPS):
                    kh, kw = divmod(t, 3)
                    lo = wb_sb[0:CIN, t * COUT : (t + 1) * COUT]
                    hi = wb_sb[CIN:128, t * COUT : (t + 1) * COUT]
                    st = t == 0
                    sp = t == N_TAPS - 1
                    # adjacent lo/hi matmuls run concurrently on disjoint
                    # PE row-group halves (different PSUM banks).  The
                    # last slab runs group 1 first so its banks finish
                    # early and the epilogue/store pipeline drains sooner.
                    pairs = [
                        (psA0, psB0, h0),
                        (psA1, psB1, h1),
                    ]
                    if last:
                        pairs.reverse()
                    for psA, psB, h in pairs:
                        nc.tensor.matmul(
                            psA[:],
                            lo,
                            X3[0:CIN, h + kh : h + kh + ROWS_PER_BANK, kw : kw + W],
                            start=st,
                            stop=sp,
                        )
                        nc.tensor.matmul(
                            psB[:],
                            hi,
                            X3[CIN:128, h + kh : h + kh + ROWS_PER_BANK, kw : kw + W],
                            start=st,
                            stop=sp,
                        )
                # bias-add into fp16 tile, layout [r(8), img(2), w(128)];
                # ScalarE takes the first 4 rows, VectorE the last 4
                ob = opool.tile([COUT, ROWS_PER_SLAB * IMGS_PER_CORE * W], f16)
                obv = ob.rearrange("p (r i c) -> p r i c", i=IMGS_PER_CORE, c=W)
                psA0v = psA0.rearrange("p (r c) -> p r c", c=W)
                psB0v = psB0.rearrange("p (r c) -> p r c", c=W)
                psA1v = psA1.rearrange("p (r c) -> p r c", c=W)
                psB1v = psB1.rearrange("p (r c) -> p r c", c=W)
                if s < H // ROWS_PER_SLAB - 1:
                    nc.scalar.add(obv[:, 0:4, 0, :], psA0v[:], b_sb)
                    nc.scalar.add(obv[:, 0:4, 1, :], psB0v[:], b_sb)
                    nc.vector.tensor_scalar_add(obv[:, 4:8, 0, :], psA1v[:], b_sb)
                    nc.vector.tensor_scalar_add(obv[:, 4:8, 1, :], psB1v[:], b_sb)
                    # one contiguous 512 KB store per slab
                    nc.scalar.dma_start(
                        out=out_d[
                            :,
                            h0 * IMGS_PER_CORE * W : (h0 + ROWS_PER_SLAB)
                            * IMGS_PER_CORE
                            * W,
                        ],
                        in_=ob[:],
                    )
                else:
                    # last slab: group 1 ran first (pairs reversed), so
                    # its banks are done ~0.4us before group 0's.  Both
                    # engines evacuate group 1 in parallel, its 256 KB
                    # store goes out on the idle sync ring, then group 0
                    # follows -- the final receipt round-trip overlaps
                    # the previous store's transfer.
                    half = ROWS_PER_BANK * IMGS_PER_CORE * W
                    nc.scalar.add(obv[:, 4:8, 0, :], psA1v[:], b_sb)
                    nc.vector.tensor_scalar_add(obv[:, 4:8, 1, :], psB1v[:], b_sb)
                    nc.sync.dma_start(
                        out=out_d[:, h0 * IMGS_PER_CORE * W + half :][:, 0:half],
                        in_=ob[:, half : 2 * half],
                    )
                    nc.scalar.add(obv[:, 0:4, 0, :], psA0v[:], b_sb)
                    nc.vector.tensor_scalar_add(obv[:, 0:4, 1, :], psB0v[:], b_sb)
                    nc.sync.dma_start(
                        out=out_d[:, h0 * IMGS_PER_CORE * W :][:, 0:half],
                        in_=ob[:, 0:half],
                    )
    nc.compile()
    return nc


def _get_nc():
    if "nc" not in _cache:
        _cache["nc"] = _build_nc()
    return _cache["nc"]


def _prepare_in_maps(input_tensor, weights, bias):
    input_tensor = np.asarray(input_tensor, dtype=np.float32)
    weights = np.asarray(weights, dtype=np.float32)
    bias = np.asarray(bias, dtype=np.float32)
    # wb[ci, t*128+co] = W[co, ci, kh, kw], t = kh*3+kw; both halves
    w9 = weights.transpose(1, 2, 3, 0).reshape(CIN, N_TAPS * COUT)  # ci,(kh kw co)
    wb = np.empty((128, N_TAPS * COUT), dtype=np.float16)
    wb[0:CIN] = w9
    wb[CIN:128] = w9
    wb = np.ascontiguousarray(wb)
    b = np.ascontiguousarray(bias.reshape(COUT, 1))
    in_maps = []
    for c in range(N_CORES):
        imgs = input_tensor[c * IMGS_PER_CORE : (c + 1) * IMGS_PER_CORE]
        zp = np.zeros((IMGS_PER_CORE, CIN, HPAD, WPAD), dtype=np.float16)
        zp[:, :, 1 : H + 1, 1 : W + 1] = imgs
        shard = np.ascontiguousarray(zp.reshape(128, HPAD * WPAD))
        in_maps.append({"x": shard, "wb": wb, "b": b})
    return in_maps


def _gather(results):
    outs = []
    for c in range(N_CORES):
        o = results[c]["out"].reshape(COUT, H, IMGS_PER_CORE, W)
        outs.append(np.ascontiguousarray(o.transpose(2, 0, 1, 3), dtype=np.float32))
    return np.concatenate(outs, axis=0)


def kernel(input_tensor, weights, bias):
    from concourse.bass_utils import run_bass_kernel_spmd

    nc = _get_nc()
    in_maps = _prepare_in_maps(input_tensor, weights, bias)
    res = run_bass_kernel_spmd(nc, in_maps, core_ids=list(range(N_CORES)))
    return _gather(res.results)
